# revision 61
# baseline (speedup 1.0000x reference)
import sys
from contextlib import ExitStack

import numpy as np

sys.path.insert(0, "/opt/trn_rl_repo")

import ml_dtypes

BF16 = ml_dtypes.bfloat16

# Problem constants (hardcoded per contract)
N_NODES = 50000
N_EDGES = 1600000
G = 32        # EDGE_FEAT
GP = G + 1    # feat cols + ones column (for segment sum S / b_e fold)
HID = 64      # EDGE_HIDDEN
H = 128       # NODE_FEAT
CORES = 8
NT = 49       # tiles (of 128 nodes) per core
NPC = NT * 128
NPAD = NPC * CORES
NEG = -10000.0  # padding logit -> exp == 0

GROUP_SIZES = [4] * 12 + [1]


def _pick_p(K):
    return max(1, min(128 // K, 8))


def _chunks(P):
    nfull, rem = 128 // P, 128 % P
    ch = [(i * P, P) for i in range(nfull)]
    if rem:
        ch.append((nfull * P, rem))
    return ch


def _pairs(T):
    if T == 1:
        return [(0,)]
    return [tuple(range(i, i + 2)) for i in range(0, T, 2)]


def _emit_order(n):
    # ascending K (gspec is descending): small groups first -> short fill
    return list(range(n - 1, -1, -1))


def _build_bass(gspec):
    """gspec: tuple of (K, T) per group; sum T == NT."""
    from concourse import bacc, mybir
    import concourse.tile as tile
    import concourse.bass_interp as _bi
    from concourse.cost_model import InstructionCostModel, as_legacy_model
    from concourse.hw_specs import get_hw_spec

    # The tile scheduler's CoreSim prices DMA with the legacy v1 model,
    # far off the timeline model's bus rate. Feed the scheduler v2 costs
    # via the on_inst_cost hook while building (restored after).
    class _V2CoreSim(_bi.CoreSim):
        def __init__(self, *a, **k):
            super().__init__(*a, **k)
            cm = InstructionCostModel(get_hw_spec(self.module.trn_type))

            def _cb(inst, d0, c0):
                try:
                    return as_legacy_model(cm.visit(inst, self))
                except Exception:
                    return (d0, c0)

            self._sim_state.on_inst_cost = _cb

    bf = mybir.dt.bfloat16

    nc_obj = bacc.Bacc(
        "TRN2", target_bir_lowering=False, debug=False,
        enable_asserts=False, num_devices=CORES,
    )

    i8 = mybir.dt.int8
    pf_ds = []
    for gi, (K, T) in enumerate(gspec):
        P = _pick_p(K)
        C = len(_chunks(P))
        PK = P * K
        pf_ds.append((
            nc_obj.dram_tensor(f"pff{gi}", [PK, T * C * GP], i8, kind="ExternalInput").ap(),
            nc_obj.dram_tensor(f"pfl{gi}", [PK, T * 128], bf, kind="ExternalInput").ap(),
        ))
    nfT_d = nc_obj.dram_tensor("nfT", [H, NPC], bf, kind="ExternalInput").ap()
    nf_d = nc_obj.dram_tensor("nf", [H, NT * H], bf, kind="ExternalInput").ap()
    consts_d = nc_obj.dram_tensor("consts", [128, 1090], bf, kind="ExternalInput").ap()
    hout_d = nc_obj.dram_tensor("hout", [H, NT * H], bf, kind="ExternalOutput").ap()

    _orig_coresim = tile.CoreSim
    tile.CoreSim = _V2CoreSim
    try:
        _build_body(nc_obj, gspec, tile, mybir, pf_ds, nfT_d, nf_d, consts_d, hout_d)
    finally:
        tile.CoreSim = _orig_coresim

    nc_obj.compile()
    return nc_obj


def _build_body(nc_obj, gspec, tile, mybir, pf_ds, nfT_d, nf_d, consts_d, hout_d):
    f32 = mybir.dt.float32
    bf = mybir.dt.bfloat16
    f8 = mybir.dt.float8e4
    AF = mybir.ActivationFunctionType
    OP = mybir.AluOpType

    order = _emit_order(len(gspec))
    # emission-order tile offsets (host lays out nfT/nf/hout in this order)
    emit_off = {}
    acc = 0
    for gi in order:
        emit_off[gi] = acc
        acc += gspec[gi][1]
    assert acc == NT

    with tile.TileContext(nc_obj) as tc, ExitStack() as ctx:
        nc = tc.nc
        cpool = ctx.enter_context(tc.tile_pool(name="consts", bufs=1))
        consts = cpool.tile([128, 1090], bf, tag="consts")
        with tc.high_priority(offset=6000):
            nc.sync.dma_start(consts[:], consts_d)
        weTa = consts[0:GP, 0:64]
        wih = consts[0:HID + 1, 64:448]
        whh = consts[:, 448:832]
        bhhn = consts[HID:HID + 1, 832:960]
        identb = consts[:, 960:1088]
        onesc = consts[:, 1088:1089]  # ones column (bf16, unused now)
        onesf8 = cpool.tile([128, 1], f8, tag="onesf8")
        nc.gpsimd.memset(onesf8[:], 1.0)

        # whole-core node features (h-major for matmuls, partition-major for
        # the elementwise tail), split early/rest so the first groups'
        # consumers don't wait on the big remainder DMA
        SPL = 9 * 128
        nfT_e = cpool.tile([H, SPL], bf, tag="nfT_e")
        nfT_r = cpool.tile([H, NPC - SPL], bf, tag="nfT_r")
        nf_e = cpool.tile([H, SPL], bf, tag="nf_e")
        nf_r = cpool.tile([H, NT * H - SPL], bf, tag="nf_r")

        def nf_view(e_buf, r_buf, c0, w):
            if c0 < SPL:
                assert c0 + w <= SPL
                return e_buf[:, c0:c0 + w]
            return r_buf[:, c0 - SPL:c0 - SPL + w]
        # output accumulator (stored by two DMAs near the end)
        ho_all = cpool.tile([H, NT * H], bf, tag="ho_all")
        # persistent ctxT buffers with a constant ones row (row HID)
        ctxTbufs = [
            cpool.tile([HID + 1, 512], bf, tag=f"ctxT{i}", name=f"ctxT{i}")
            for i in range(2)
        ]
        for t_ in ctxTbufs:
            nc.gpsimd.memset(t_[HID:HID + 1, :], 1.0)

        inp = ctx.enter_context(tc.tile_pool(name="inp", bufs=2))
        mid = ctx.enter_context(tc.tile_pool(name="mid", bufs=2))
        pY = ctx.enter_context(tc.tile_pool(name="pY", bufs=2, space="PSUM"))
        pC = ctx.enter_context(tc.tile_pool(name="pC", bufs=1, space="PSUM"))
        pX = ctx.enter_context(tc.tile_pool(name="pX", bufs=2, space="PSUM"))
        pR = ctx.enter_context(tc.tile_pool(name="pR", bufs=2, space="PSUM"))
        pN = ctx.enter_context(tc.tile_pool(name="pN", bufs=2, space="PSUM"))

        def stageA(gi):
            K, T = gspec[gi]
            P = _pick_p(K)
            ch = _chunks(P)
            C = len(ch)
            PK = P * K
    
            pff8 = inp.tile([128, T * C * GP], mybir.dt.int8, tag="pff")
            nc.sync.dma_start(pff8[0:PK, :], pf_ds[gi][0])
            pff = pff8[:].bitcast(f8)
            pfl = inp.tile([128, T * 128], bf, tag="pfl")
            nc.sync.dma_start(pfl[0:PK, :], pf_ds[gi][1])
            return dict(gi=gi, K=K, T=T, P=P, ch=ch, C=C, PK=PK, pff=pff, pfl=pfl)

        def stageB(st):
            K, T, ch, C, PK = st["K"], st["T"], st["ch"], st["C"], st["PK"]
            pff, pfl = st["pff"], st["pfl"]
            ex = mid.tile([128, T * 128], f8, tag="ex")
            nc.scalar.activation(ex[0:PK, :], pfl[0:PK, :], AF.Exp)
            y_ps = pY.tile([GP, T * 128], f32, tag="y")
            for t in range(T):
                for c, (cs, pc) in enumerate(ch):
                    nc.tensor.matmul(
                        y_ps[:, t * 128 + cs: t * 128 + cs + pc],
                        pff[0:pc * K, t * C * GP + c * GP: t * C * GP + (c + 1) * GP],
                        ex[0:pc * K, t * 128 + cs: t * 128 + cs + pc],
                        start=True, stop=True,
                    )
            # S per node, node-partition layout: EX_t^T @ ones
            # (borrows the tail columns of the cr bank; written in B, cr in C1)
            crst = pC.tile([H, T * HID + 4], f32, tag="cr")
            st["crst"] = crst
            sT_ps = crst[:, T * HID:T * HID + T]
            for t in range(T):
                nc.tensor.matmul(
                    sT_ps[:, t:t + 1],
                    ex[0:PK, t * 128:(t + 1) * 128],
                    onesf8[0:PK, :],
                    start=True, stop=True,
                )
            y_sb = mid.tile([GP, T * 128], bf, tag="ysb")
            nc.scalar.activation(y_sb[:], y_ps[:], AF.Copy)
            rS = mid.tile([H, T], f32, tag="rS")
            nc.vector.reciprocal(rS[:], sT_ps[:])
            st.update(y_sb=y_sb, rS=rS)
            return st

        def stageC1(st):
            gi, T = st["gi"], st["T"]
            y_sb, rS = st["y_sb"], st["rS"]
            TH = T * 128
            # c_raw[node, :] = y_t^T @ [W_e^T; b_e]  (+= S*b_e via ones col)
            cr_ps = st["crst"][:, 0:T * HID]
            for t in range(T):
                nc.tensor.matmul(
                    cr_ps[:, t * HID:(t + 1) * HID],
                    y_sb[:, t * 128:(t + 1) * 128], weTa,
                    start=True, stop=True,
                )
            # x = c_raw / S (+ b_e); ctx' = elu(x)+1 = relu(x) + min(exp(x),1)
            x = mid.tile([H, T * HID], bf, tag="x")
            rSb = (
                rS[:].rearrange("p (t o) -> p t o", o=1)
                .broadcast_to([H, T, HID])
            )
            nc.vector.tensor_tensor(
                x[:].rearrange("p (t c) -> p t c", c=HID),
                cr_ps[:].rearrange("p (t c) -> p t c", c=HID),
                rSb, op=OP.mult,
            )
            ev = mid.tile([H, T * HID], bf, tag="ev")
            nc.scalar.activation(ev[:], x[:], AF.Exp)
            rn = mid.tile([H, T * HID], bf, tag="rn")
            nc.gpsimd.tensor_scalar_max(rn[:], x[:], 0.0)
            em = mid.tile([H, T * HID], bf, tag="em")
            nc.gpsimd.tensor_scalar_min(em[:], ev[:], 1.0)
            ctxn = mid.tile([H, T * HID], bf, tag="ctxn")
            nc.vector.tensor_tensor(ctxn[:], em[:], rn[:], op=OP.add)
            # transpose ctx to [HID, node] (ones row persistent in ctxT buf)
            ctxT_ps = pX.tile([HID, TH], bf, tag="ctxTp", bufs=1)
            for t in range(T):
                nc.tensor.transpose(
                    ctxT_ps[:, t * 128:(t + 1) * 128],
                    ctxn[:, t * HID:(t + 1) * HID],
                    identb,
                )
            ctxT = ctxTbufs[st["buf"]]
            nc.vector.tensor_copy(ctxT[0:HID, 0:TH], ctxT_ps[:])
            st["ctxT"] = ctxT
            return st

        def stageC2(st):
            gi, T = st["gi"], st["T"]
            c0 = emit_off[gi] * 128
            TH = T * 128
            ctxT = st["ctxT"]

            # --- GRU gates ---
            t2 = mid.tile([H, TH], bf, tag="t2")
            zq = mid.tile([H, TH], bf, tag="zq")
            sigq = mid.tile([H, 2 * TH], bf, tag="sigq")
            for pair in _pairs(T):
                P2 = len(pair)
                p0 = pair[0]
                rz_ps = pR.tile([H, P2 * 2 * H], f32, tag="rz")
                inhn_ps = pN.tile([H, P2 * 2 * H], f32, tag="inhn")
                for ti, t in enumerate(pair):
                    ctx_t = ctxT[:, t * H:(t + 1) * H]
                    nfT_c = nf_view(nfT_e, nfT_r, c0 + t * H, H)
                    nc.tensor.matmul(
                        rz_ps[:, ti * 2 * H:(ti + 1) * 2 * H],
                        ctx_t, wih[:, 0:2 * H], start=True, stop=False,
                    )
                    nc.tensor.matmul(
                        rz_ps[:, ti * 2 * H:(ti + 1) * 2 * H],
                        nfT_c, whh[:, 0:2 * H], start=False, stop=True,
                    )
                    # in_n = ctx@wih_n (+bias via ones row of ctxT)
                    nc.tensor.matmul(
                        inhn_ps[:, ti * 2 * H:ti * 2 * H + H],
                        ctx_t, wih[:, 2 * H:], start=True, stop=True,
                    )
                    # hn = 0.5*(nf@whh_n) + 0.5*bhh_n
                    nc.tensor.matmul(
                        inhn_ps[:, ti * 2 * H + H:(ti + 1) * 2 * H],
                        nfT_c, whh[:, 2 * H:], start=True, stop=False,
                    )
                    nc.tensor.matmul(
                        inhn_ps[:, ti * 2 * H + H:(ti + 1) * 2 * H],
                        ctxT[HID:HID + 1, t * H:(t + 1) * H], bhhn,
                        start=False, stop=True,
                    )
                # trz = tanh(0.5 * rz); r = (1+tr)/2, z = (1+tz)/2
                sig = sigq[:, p0 * 2 * H:(p0 + P2) * 2 * H]
                nc.scalar.activation(sig, rz_ps[:], AF.Tanh, scale=0.5)
                sigv = sig.rearrange("p (t two h) -> p t two h", two=2, h=H)
                tr_v = sigv[:, :, 0, :]
                # z = 0.5*tz + 0.5 (true sigmoid), off the critical chain
                nc.vector.tensor_scalar(
                    zq[:, p0 * H:(p0 + P2) * H].rearrange("p (t h) -> p t h", h=H),
                    sigv[:, :, 1, :], 0.5, 0.5, op0=OP.mult, op1=OP.add,
                )
                iv = inhn_ps[:].rearrange("p (t two h) -> p t two h", two=2, h=H)
                in_v = iv[:, :, 0, :]
                hn_v = iv[:, :, 1, :]
                # t2 = (tr+1)*hn + in_n  (= i_n + r*(h_n + bhh_n))
                t1 = mid.tile([H, P2 * H], bf, tag="t1")
                nc.vector.scalar_tensor_tensor(
                    t1[:].rearrange("p (t h) -> p t h", h=H),
                    tr_v, 1.0, hn_v, OP.add, OP.mult,
                )
                nc.vector.tensor_tensor(
                    t2[:, p0 * H:(p0 + P2) * H].rearrange("p (t h) -> p t h", h=H),
                    t1[:].rearrange("p (t h) -> p t h", h=H),
                    in_v, op=OP.add,
                )
            # n = tanh(t2); h = relu(n + z*(nf - n))
            n_t = mid.tile([H, TH], bf, tag="n")
            nc.scalar.activation(n_t[:], t2[:], AF.Tanh)
            nf_v = nf_view(nf_e, nf_r, c0, TH)
            d_t = mid.tile([H, TH], bf, tag="d")
            nc.vector.tensor_tensor(d_t[:], nf_v, n_t[:], op=OP.subtract)
            zd = mid.tile([H, TH], bf, tag="zd")
            nc.vector.tensor_tensor(zd[:], zq[:, 0:TH], d_t[:], op=OP.mult)
            s1 = mid.tile([H, TH], bf, tag="s1")
            nc.vector.tensor_tensor(s1[:], n_t[:], zd[:], op=OP.add)
            nc.vector.tensor_scalar_max(ho_all[:, c0:c0 + TH], s1[:], 0.0)

        # --- 4-stage software pipeline over groups:
        # A(g) dma | B(g-1) exp+y | C1(g-2) ctx | C2(g-3) gru ---
        NG = len(gspec)
        stA = stB = stC = None
        bufc = 0
        consumed = 0
        store0_cols = None
        store1_cols = None
        for i in range(NG + 3):
            a = None
            if i < NG:
                with tc.high_priority(offset=1200):
                    a = stageA(order[i])
            if stA is not None:
                with tc.high_priority(offset=600):
                    newB = stageB(stA)
            else:
                newB = None
            if i == 0:
                # node features for the first few emitted groups right after
                # the first pf group; the big remainder rides mid-stream
                with tc.high_priority(offset=5000):
                    nc.sync.dma_start(nfT_e[:], nfT_d[:, 0:SPL])
                    nc.sync.dma_start(nf_e[:], nf_d[:, 0:SPL])
            if i == 4:
                nc.sync.dma_start(nfT_r[:], nfT_d[:, SPL:])
                nc.sync.dma_start(nf_r[:], nf_d[:, SPL:])
            if stB is not None:
                stB["buf"] = bufc % 2
                bufc += 1
                newC = stageC1(stB)
            else:
                newC = None
            if stC is not None:
                stageC2(stC)
                consumed += stC["T"]
            stA, stB, stC = a, newB, newC
            if i == NG - 2 and store0_cols is None:
                store0_cols = consumed * 128
                nc.sync.dma_start(
                    hout_d[:, 0:store0_cols], ho_all[:, 0:store0_cols]
                )
            if i == NG + 1 and store1_cols is None:
                # partial store once all but the last two groups' tails are
                # queued; covers everything already computed
                store1_cols = consumed * 128
                nc.sync.dma_start(
                    hout_d[:, store0_cols:store1_cols],
                    ho_all[:, store0_cols:store1_cols],
                )
        nc.sync.dma_start(
            hout_d[:, store1_cols:], ho_all[:, store1_cols:]
        )
        assert consumed == NT


# revision 62
# speedup vs baseline: 1.0057x; 1.0057x over previous
import sys
from contextlib import ExitStack

import numpy as np

sys.path.insert(0, "/opt/trn_rl_repo")

import ml_dtypes

BF16 = ml_dtypes.bfloat16

# Problem constants (hardcoded per contract)
N_NODES = 50000
N_EDGES = 1600000
G = 32        # EDGE_FEAT
GP = G + 1    # feat cols + ones column (for segment sum S / b_e fold)
HID = 64      # EDGE_HIDDEN
H = 128       # NODE_FEAT
CORES = 8
NT = 49       # tiles (of 128 nodes) per core
NPC = NT * 128
NPAD = NPC * CORES
NEG = -10000.0  # padding logit -> exp == 0

GROUP_SIZES = [4] * 12 + [1]


def _pick_p(K):
    return max(1, min(128 // K, 8))


def _chunks(P):
    nfull, rem = 128 // P, 128 % P
    ch = [(i * P, P) for i in range(nfull)]
    if rem:
        ch.append((nfull * P, rem))
    return ch


def _pairs(T):
    if T == 1:
        return [(0,)]
    return [tuple(range(i, i + 2)) for i in range(0, T, 2)]


def _emit_order(n):
    # ascending K (gspec is descending): small groups first -> short fill
    return list(range(n - 1, -1, -1))


def _build_bass(gspec):
    """gspec: tuple of (K, T) per group; sum T == NT."""
    from concourse import bacc, mybir
    import concourse.tile as tile
    import concourse.bass_interp as _bi
    from concourse.cost_model import InstructionCostModel, as_legacy_model
    from concourse.hw_specs import get_hw_spec

    # The tile scheduler's CoreSim prices DMA with the legacy v1 model,
    # far off the timeline model's bus rate. Feed the scheduler v2 costs
    # via the on_inst_cost hook while building (restored after).
    class _V2CoreSim(_bi.CoreSim):
        def __init__(self, *a, **k):
            super().__init__(*a, **k)
            cm = InstructionCostModel(get_hw_spec(self.module.trn_type))

            def _cb(inst, d0, c0):
                try:
                    return as_legacy_model(cm.visit(inst, self))
                except Exception:
                    return (d0, c0)

            self._sim_state.on_inst_cost = _cb

    bf = mybir.dt.bfloat16

    nc_obj = bacc.Bacc(
        "TRN2", target_bir_lowering=False, debug=False,
        enable_asserts=False, num_devices=CORES,
    )

    i8 = mybir.dt.int8
    pf_ds = []
    for gi, (K, T) in enumerate(gspec):
        P = _pick_p(K)
        C = len(_chunks(P))
        PK = P * K
        pf_ds.append((
            nc_obj.dram_tensor(f"pff{gi}", [PK, T * C * GP], i8, kind="ExternalInput").ap(),
            nc_obj.dram_tensor(f"pfl{gi}", [PK, T * 128], bf, kind="ExternalInput").ap(),
        ))
    nfT_d = nc_obj.dram_tensor("nfT", [H, NPC], bf, kind="ExternalInput").ap()
    nf_d = nc_obj.dram_tensor("nf", [H, NT * H], bf, kind="ExternalInput").ap()
    consts_d = nc_obj.dram_tensor("consts", [128, 1090], bf, kind="ExternalInput").ap()
    hout_d = nc_obj.dram_tensor("hout", [H, NT * H], bf, kind="ExternalOutput").ap()

    _orig_coresim = tile.CoreSim
    tile.CoreSim = _V2CoreSim
    try:
        _build_body(nc_obj, gspec, tile, mybir, pf_ds, nfT_d, nf_d, consts_d, hout_d)
    finally:
        tile.CoreSim = _orig_coresim

    nc_obj.compile()
    return nc_obj


def _build_body(nc_obj, gspec, tile, mybir, pf_ds, nfT_d, nf_d, consts_d, hout_d):
    f32 = mybir.dt.float32
    bf = mybir.dt.bfloat16
    f8 = mybir.dt.float8e4
    AF = mybir.ActivationFunctionType
    OP = mybir.AluOpType

    order = _emit_order(len(gspec))
    # emission-order tile offsets (host lays out nfT/nf/hout in this order)
    emit_off = {}
    acc = 0
    for gi in order:
        emit_off[gi] = acc
        acc += gspec[gi][1]
    assert acc == NT

    with tile.TileContext(nc_obj) as tc, ExitStack() as ctx:
        nc = tc.nc
        cpool = ctx.enter_context(tc.tile_pool(name="consts", bufs=1))
        consts = cpool.tile([128, 1090], bf, tag="consts")
        with tc.high_priority(offset=6000):
            nc.sync.dma_start(consts[:], consts_d)
        weTa = consts[0:GP, 0:64]
        wih = consts[0:HID + 1, 64:448]
        whh = consts[:, 448:832]
        bhhn = consts[HID:HID + 1, 832:960]
        identb = consts[:, 960:1088]
        onesc = consts[:, 1088:1089]  # ones column (bf16, unused now)
        onesf8 = cpool.tile([128, 1], f8, tag="onesf8")
        nc.gpsimd.memset(onesf8[:], 1.0)

        # whole-core node features (h-major for matmuls, partition-major for
        # the elementwise tail), split early/rest so the first groups'
        # consumers don't wait on the big remainder DMA
        SPL = 9 * 128
        nfT_e = cpool.tile([H, SPL], bf, tag="nfT_e")
        nfT_r = cpool.tile([H, NPC - SPL], bf, tag="nfT_r")
        nf_e = cpool.tile([H, SPL], bf, tag="nf_e")
        nf_r = cpool.tile([H, NT * H - SPL], bf, tag="nf_r")

        def nf_view(e_buf, r_buf, c0, w):
            if c0 < SPL:
                assert c0 + w <= SPL
                return e_buf[:, c0:c0 + w]
            return r_buf[:, c0 - SPL:c0 - SPL + w]
        # output accumulator (stored by two DMAs near the end)
        ho_all = cpool.tile([H, NT * H], bf, tag="ho_all")
        # persistent ctxT buffers with a constant ones row (row HID)
        ctxTbufs = [
            cpool.tile([HID + 1, 512], bf, tag=f"ctxT{i}", name=f"ctxT{i}")
            for i in range(2)
        ]
        for t_ in ctxTbufs:
            nc.gpsimd.memset(t_[HID:HID + 1, :], 1.0)

        inp = ctx.enter_context(tc.tile_pool(name="inp", bufs=2))
        mid = ctx.enter_context(tc.tile_pool(name="mid", bufs=2))
        pY = ctx.enter_context(tc.tile_pool(name="pY", bufs=1, space="PSUM"))
        pS = ctx.enter_context(tc.tile_pool(name="pS", bufs=1, space="PSUM"))
        pC = ctx.enter_context(tc.tile_pool(name="pC", bufs=1, space="PSUM"))
        pX = ctx.enter_context(tc.tile_pool(name="pX", bufs=2, space="PSUM"))
        pR = ctx.enter_context(tc.tile_pool(name="pR", bufs=2, space="PSUM"))
        pN = ctx.enter_context(tc.tile_pool(name="pN", bufs=2, space="PSUM"))

        def stageA(gi):
            K, T = gspec[gi]
            P = _pick_p(K)
            ch = _chunks(P)
            C = len(ch)
            PK = P * K
    
            pff8 = inp.tile([128, T * C * GP], mybir.dt.int8, tag="pff")
            nc.sync.dma_start(pff8[0:PK, :], pf_ds[gi][0])
            pff = pff8[:].bitcast(f8)
            pfl = inp.tile([128, T * 128], bf, tag="pfl")
            nc.sync.dma_start(pfl[0:PK, :], pf_ds[gi][1])
            return dict(gi=gi, K=K, T=T, P=P, ch=ch, C=C, PK=PK, pff=pff, pfl=pfl)

        def stageB(st):
            K, T, ch, C, PK = st["K"], st["T"], st["ch"], st["C"], st["PK"]
            pff, pfl = st["pff"], st["pfl"]
            ex = mid.tile([128, T * 128], f8, tag="ex")
            nc.scalar.activation(ex[0:PK, :], pfl[0:PK, :], AF.Exp)
            y_ps = pY.tile([GP, T * 128], f32, tag="y")
            for t in range(T):
                for c, (cs, pc) in enumerate(ch):
                    nc.tensor.matmul(
                        y_ps[:, t * 128 + cs: t * 128 + cs + pc],
                        pff[0:pc * K, t * C * GP + c * GP: t * C * GP + (c + 1) * GP],
                        ex[0:pc * K, t * 128 + cs: t * 128 + cs + pc],
                        start=True, stop=True,
                    )
            # S per node, node-partition layout: EX_t^T @ ones
            sT_ps = pS.tile([H, T], f32, tag="sT")
            for t in range(T):
                nc.tensor.matmul(
                    sT_ps[:, t:t + 1],
                    ex[0:PK, t * 128:(t + 1) * 128],
                    onesf8[0:PK, :],
                    start=True, stop=True,
                )
            y_sb = mid.tile([GP, T * 128], bf, tag="ysb")
            nc.scalar.activation(y_sb[:], y_ps[:], AF.Copy)
            rS = mid.tile([H, T], f32, tag="rS")
            nc.vector.reciprocal(rS[:], sT_ps[:])
            st.update(y_sb=y_sb, rS=rS)
            return st

        def stageC1(st):
            gi, T = st["gi"], st["T"]
            y_sb, rS = st["y_sb"], st["rS"]
            TH = T * 128
            # c_raw[node, :] = y_t^T @ [W_e^T; b_e]  (+= S*b_e via ones col)
            cr_ps = pC.tile([H, T * HID], f32, tag="cr")
            for t in range(T):
                nc.tensor.matmul(
                    cr_ps[:, t * HID:(t + 1) * HID],
                    y_sb[:, t * 128:(t + 1) * 128], weTa,
                    start=True, stop=True,
                )
            # x = c_raw / S (+ b_e); ctx' = elu(x)+1 = relu(x) + min(exp(x),1)
            x = mid.tile([H, T * HID], bf, tag="x")
            rSb = (
                rS[:].rearrange("p (t o) -> p t o", o=1)
                .broadcast_to([H, T, HID])
            )
            nc.vector.tensor_tensor(
                x[:].rearrange("p (t c) -> p t c", c=HID),
                cr_ps[:].rearrange("p (t c) -> p t c", c=HID),
                rSb, op=OP.mult,
            )
            ev = mid.tile([H, T * HID], bf, tag="ev")
            nc.scalar.activation(ev[:], x[:], AF.Exp)
            rn = mid.tile([H, T * HID], bf, tag="rn")
            nc.gpsimd.tensor_scalar_max(rn[:], x[:], 0.0)
            em = mid.tile([H, T * HID], bf, tag="em")
            nc.gpsimd.tensor_scalar_min(em[:], ev[:], 1.0)
            ctxn = mid.tile([H, T * HID], bf, tag="ctxn")
            nc.vector.tensor_tensor(ctxn[:], em[:], rn[:], op=OP.add)
            # transpose ctx to [HID, node] (ones row persistent in ctxT buf)
            ctxT_ps = pX.tile([HID, TH], bf, tag="ctxTp", bufs=1)
            for t in range(T):
                nc.tensor.transpose(
                    ctxT_ps[:, t * 128:(t + 1) * 128],
                    ctxn[:, t * HID:(t + 1) * HID],
                    identb,
                )
            ctxT = ctxTbufs[st["buf"]]
            nc.vector.tensor_copy(ctxT[0:HID, 0:TH], ctxT_ps[:])
            st["ctxT"] = ctxT
            return st

        def stageC2(st):
            gi, T = st["gi"], st["T"]
            c0 = emit_off[gi] * 128
            TH = T * 128
            ctxT = st["ctxT"]

            # --- GRU gates ---
            t2 = mid.tile([H, TH], bf, tag="t2")
            zq = mid.tile([H, TH], bf, tag="zq")
            sigq = mid.tile([H, 2 * TH], bf, tag="sigq")
            for pair in _pairs(T):
                P2 = len(pair)
                p0 = pair[0]
                rz_ps = pR.tile([H, P2 * 2 * H], f32, tag="rz")
                inhn_ps = pN.tile([H, P2 * 2 * H], f32, tag="inhn")
                for ti, t in enumerate(pair):
                    ctx_t = ctxT[:, t * H:(t + 1) * H]
                    nfT_c = nf_view(nfT_e, nfT_r, c0 + t * H, H)
                    nc.tensor.matmul(
                        rz_ps[:, ti * 2 * H:(ti + 1) * 2 * H],
                        ctx_t, wih[:, 0:2 * H], start=True, stop=False,
                    )
                    nc.tensor.matmul(
                        rz_ps[:, ti * 2 * H:(ti + 1) * 2 * H],
                        nfT_c, whh[:, 0:2 * H], start=False, stop=True,
                    )
                    # in_n = ctx@wih_n (+bias via ones row of ctxT)
                    nc.tensor.matmul(
                        inhn_ps[:, ti * 2 * H:ti * 2 * H + H],
                        ctx_t, wih[:, 2 * H:], start=True, stop=True,
                    )
                    # hn = 0.5*(nf@whh_n) + 0.5*bhh_n
                    nc.tensor.matmul(
                        inhn_ps[:, ti * 2 * H + H:(ti + 1) * 2 * H],
                        nfT_c, whh[:, 2 * H:], start=True, stop=False,
                    )
                    nc.tensor.matmul(
                        inhn_ps[:, ti * 2 * H + H:(ti + 1) * 2 * H],
                        ctxT[HID:HID + 1, t * H:(t + 1) * H], bhhn,
                        start=False, stop=True,
                    )
                # trz = tanh(0.5 * rz); r = (1+tr)/2, z = (1+tz)/2
                sig = sigq[:, p0 * 2 * H:(p0 + P2) * 2 * H]
                nc.scalar.activation(sig, rz_ps[:], AF.Tanh, scale=0.5)
                sigv = sig.rearrange("p (t two h) -> p t two h", two=2, h=H)
                tr_v = sigv[:, :, 0, :]
                # z = 0.5*tz + 0.5 (true sigmoid), off the critical chain
                nc.vector.tensor_scalar(
                    zq[:, p0 * H:(p0 + P2) * H].rearrange("p (t h) -> p t h", h=H),
                    sigv[:, :, 1, :], 0.5, 0.5, op0=OP.mult, op1=OP.add,
                )
                iv = inhn_ps[:].rearrange("p (t two h) -> p t two h", two=2, h=H)
                in_v = iv[:, :, 0, :]
                hn_v = iv[:, :, 1, :]
                # t2 = (tr+1)*hn + in_n  (= i_n + r*(h_n + bhh_n))
                t1 = mid.tile([H, P2 * H], bf, tag="t1")
                nc.vector.scalar_tensor_tensor(
                    t1[:].rearrange("p (t h) -> p t h", h=H),
                    tr_v, 1.0, hn_v, OP.add, OP.mult,
                )
                nc.vector.tensor_tensor(
                    t2[:, p0 * H:(p0 + P2) * H].rearrange("p (t h) -> p t h", h=H),
                    t1[:].rearrange("p (t h) -> p t h", h=H),
                    in_v, op=OP.add,
                )
            # n = tanh(t2); h = relu(n + z*(nf - n))
            n_t = mid.tile([H, TH], bf, tag="n")
            nc.scalar.activation(n_t[:], t2[:], AF.Tanh)
            nf_v = nf_view(nf_e, nf_r, c0, TH)
            d_t = mid.tile([H, TH], bf, tag="d")
            nc.vector.tensor_tensor(d_t[:], nf_v, n_t[:], op=OP.subtract)
            zd = mid.tile([H, TH], bf, tag="zd")
            nc.vector.tensor_tensor(zd[:], zq[:, 0:TH], d_t[:], op=OP.mult)
            s1 = mid.tile([H, TH], bf, tag="s1")
            nc.vector.tensor_tensor(s1[:], n_t[:], zd[:], op=OP.add)
            nc.vector.tensor_scalar_max(ho_all[:, c0:c0 + TH], s1[:], 0.0)

        # --- 4-stage software pipeline over groups:
        # A(g) dma | B(g-1) exp+y | C1(g-2) ctx | C2(g-3) gru ---
        NG = len(gspec)
        stA = stB = stC = None
        bufc = 0
        consumed = 0
        store0_cols = None
        store1_cols = None
        for i in range(NG + 3):
            a = None
            if i < NG:
                with tc.high_priority(offset=1200):
                    a = stageA(order[i])
            if stA is not None:
                with tc.high_priority(offset=600):
                    newB = stageB(stA)
            else:
                newB = None
            if i == 0:
                # node features for the first few emitted groups right after
                # the first pf group; the big remainder rides mid-stream
                with tc.high_priority(offset=5000):
                    nc.sync.dma_start(nfT_e[:], nfT_d[:, 0:SPL])
                    nc.sync.dma_start(nf_e[:], nf_d[:, 0:SPL])
            if i == 4:
                nc.sync.dma_start(nfT_r[:], nfT_d[:, SPL:])
                nc.sync.dma_start(nf_r[:], nf_d[:, SPL:])
            if stB is not None:
                stB["buf"] = bufc % 2
                bufc += 1
                newC = stageC1(stB)
            else:
                newC = None
            if stC is not None:
                stageC2(stC)
                consumed += stC["T"]
            stA, stB, stC = a, newB, newC
            if i == NG - 2 and store0_cols is None:
                store0_cols = consumed * 128
                nc.sync.dma_start(
                    hout_d[:, 0:store0_cols], ho_all[:, 0:store0_cols]
                )
            if i == NG + 1 and store1_cols is None:
                # partial store once all but the last two groups' tails are
                # queued; covers everything already computed
                store1_cols = consumed * 128
                nc.sync.dma_start(
                    hout_d[:, store0_cols:store1_cols],
                    ho_all[:, store0_cols:store1_cols],
                )
        nc.sync.dma_start(
            hout_d[:, store1_cols:], ho_all[:, store1_cols:]
        )
        assert consumed == NT


# revision 63
# speedup vs baseline: 1.0057x; 1.0001x over previous
import sys
from contextlib import ExitStack

import numpy as np

sys.path.insert(0, "/opt/trn_rl_repo")

import ml_dtypes

BF16 = ml_dtypes.bfloat16

# Problem constants (hardcoded per contract)
N_NODES = 50000
N_EDGES = 1600000
G = 32        # EDGE_FEAT
GP = G + 1    # feat cols + ones column (for segment sum S / b_e fold)
HID = 64      # EDGE_HIDDEN
H = 128       # NODE_FEAT
CORES = 8
NT = 49       # tiles (of 128 nodes) per core
NPC = NT * 128
NPAD = NPC * CORES
NEG = -10000.0  # padding logit -> exp == 0

GROUP_SIZES = [4] * 12 + [1]


def _pick_p(K):
    return max(1, min(128 // K, 8))


def _chunks(P):
    nfull, rem = 128 // P, 128 % P
    ch = [(i * P, P) for i in range(nfull)]
    if rem:
        ch.append((nfull * P, rem))
    return ch


def _pairs(T):
    if T == 1:
        return [(0,)]
    return [tuple(range(i, i + 2)) for i in range(0, T, 2)]


def _emit_order(n):
    # ascending K (gspec is descending): small groups first -> short fill
    return list(range(n - 1, -1, -1))


def _build_bass(gspec):
    """gspec: tuple of (K, T) per group; sum T == NT."""
    from concourse import bacc, mybir
    import concourse.tile as tile
    import concourse.bass_interp as _bi
    from concourse.cost_model import InstructionCostModel, as_legacy_model
    from concourse.hw_specs import get_hw_spec

    # The tile scheduler's CoreSim prices DMA with the legacy v1 model,
    # far off the timeline model's bus rate. Feed the scheduler v2 costs
    # via the on_inst_cost hook while building (restored after).
    class _V2CoreSim(_bi.CoreSim):
        def __init__(self, *a, **k):
            super().__init__(*a, **k)
            cm = InstructionCostModel(get_hw_spec(self.module.trn_type))

            def _cb(inst, d0, c0):
                try:
                    return as_legacy_model(cm.visit(inst, self))
                except Exception:
                    return (d0, c0)

            self._sim_state.on_inst_cost = _cb

    bf = mybir.dt.bfloat16

    nc_obj = bacc.Bacc(
        "TRN2", target_bir_lowering=False, debug=False,
        enable_asserts=False, num_devices=CORES,
    )

    i8 = mybir.dt.int8
    pf_ds = []
    for gi, (K, T) in enumerate(gspec):
        P = _pick_p(K)
        C = len(_chunks(P))
        PK = P * K
        pf_ds.append((
            nc_obj.dram_tensor(f"pff{gi}", [PK, T * C * GP], i8, kind="ExternalInput").ap(),
            nc_obj.dram_tensor(f"pfl{gi}", [PK, T * 128], bf, kind="ExternalInput").ap(),
        ))
    nfT_d = nc_obj.dram_tensor("nfT", [H, NPC], bf, kind="ExternalInput").ap()
    nf_d = nc_obj.dram_tensor("nf", [H, NT * H], bf, kind="ExternalInput").ap()
    consts_d = nc_obj.dram_tensor("consts", [128, 1090], bf, kind="ExternalInput").ap()
    hout_d = nc_obj.dram_tensor("hout", [H, NT * H], bf, kind="ExternalOutput").ap()

    _orig_coresim = tile.CoreSim
    tile.CoreSim = _V2CoreSim
    try:
        _build_body(nc_obj, gspec, tile, mybir, pf_ds, nfT_d, nf_d, consts_d, hout_d)
    finally:
        tile.CoreSim = _orig_coresim

    nc_obj.compile()
    return nc_obj


def _build_body(nc_obj, gspec, tile, mybir, pf_ds, nfT_d, nf_d, consts_d, hout_d):
    f32 = mybir.dt.float32
    bf = mybir.dt.bfloat16
    f8 = mybir.dt.float8e4
    AF = mybir.ActivationFunctionType
    OP = mybir.AluOpType

    order = _emit_order(len(gspec))
    # emission-order tile offsets (host lays out nfT/nf/hout in this order)
    emit_off = {}
    acc = 0
    for gi in order:
        emit_off[gi] = acc
        acc += gspec[gi][1]
    assert acc == NT

    with tile.TileContext(nc_obj) as tc, ExitStack() as ctx:
        nc = tc.nc
        cpool = ctx.enter_context(tc.tile_pool(name="consts", bufs=1))
        consts = cpool.tile([128, 1090], bf, tag="consts")
        with tc.high_priority(offset=6000):
            nc.sync.dma_start(consts[:], consts_d)
        weTa = consts[0:GP, 0:64]
        wih = consts[0:HID + 1, 64:448]
        whh = consts[:, 448:832]
        bhhn = consts[HID:HID + 1, 832:960]
        identb = consts[:, 960:1088]
        onesc = consts[:, 1088:1089]  # ones column (bf16, unused now)
        onesf8 = cpool.tile([128, 1], f8, tag="onesf8")
        nc.gpsimd.memset(onesf8[:], 1.0)

        # whole-core node features (h-major for matmuls, partition-major for
        # the elementwise tail), split early/rest so the first groups'
        # consumers don't wait on the big remainder DMA
        SPL = 9 * 128
        nfT_e = cpool.tile([H, SPL], bf, tag="nfT_e")
        nfT_r = cpool.tile([H, NPC - SPL], bf, tag="nfT_r")
        nf_e = cpool.tile([H, SPL], bf, tag="nf_e")
        nf_r = cpool.tile([H, NT * H - SPL], bf, tag="nf_r")

        def nf_view(e_buf, r_buf, c0, w):
            if c0 < SPL:
                assert c0 + w <= SPL
                return e_buf[:, c0:c0 + w]
            return r_buf[:, c0 - SPL:c0 - SPL + w]
        # output accumulator (stored by two DMAs near the end)
        ho_all = cpool.tile([H, NT * H], bf, tag="ho_all")
        # persistent ctxT buffers with a constant ones row (row HID)
        ctxTbufs = [
            cpool.tile([HID + 1, 512], bf, tag=f"ctxT{i}", name=f"ctxT{i}")
            for i in range(2)
        ]
        for t_ in ctxTbufs:
            nc.gpsimd.memset(t_[HID:HID + 1, :], 1.0)

        inp = ctx.enter_context(tc.tile_pool(name="inp", bufs=2))
        mid = ctx.enter_context(tc.tile_pool(name="mid", bufs=2))
        pY = ctx.enter_context(tc.tile_pool(name="pY", bufs=1, space="PSUM"))
        pS = ctx.enter_context(tc.tile_pool(name="pS", bufs=1, space="PSUM"))
        pC = ctx.enter_context(tc.tile_pool(name="pC", bufs=1, space="PSUM"))
        pX = ctx.enter_context(tc.tile_pool(name="pX", bufs=2, space="PSUM"))
        pR = ctx.enter_context(tc.tile_pool(name="pR", bufs=2, space="PSUM"))
        pN = ctx.enter_context(tc.tile_pool(name="pN", bufs=2, space="PSUM"))

        def stageA(gi):
            K, T = gspec[gi]
            P = _pick_p(K)
            ch = _chunks(P)
            C = len(ch)
            PK = P * K
    
            pff8 = inp.tile([128, T * C * GP], mybir.dt.int8, tag="pff")
            nc.sync.dma_start(pff8[0:PK, :], pf_ds[gi][0])
            pff = pff8[:].bitcast(f8)
            pfl = inp.tile([128, T * 128], bf, tag="pfl")
            nc.sync.dma_start(pfl[0:PK, :], pf_ds[gi][1])
            return dict(gi=gi, K=K, T=T, P=P, ch=ch, C=C, PK=PK, pff=pff, pfl=pfl)

        def stageB(st):
            K, T, ch, C, PK = st["K"], st["T"], st["ch"], st["C"], st["PK"]
            pff, pfl = st["pff"], st["pfl"]
            ex = mid.tile([128, T * 128], f8, tag="ex")
            nc.scalar.activation(ex[0:PK, :], pfl[0:PK, :], AF.Exp)
            y_ps = pY.tile([GP, T * 128], f32, tag="y")
            for t in range(T):
                for c, (cs, pc) in enumerate(ch):
                    nc.tensor.matmul(
                        y_ps[:, t * 128 + cs: t * 128 + cs + pc],
                        pff[0:pc * K, t * C * GP + c * GP: t * C * GP + (c + 1) * GP],
                        ex[0:pc * K, t * 128 + cs: t * 128 + cs + pc],
                        start=True, stop=True,
                    )
            # S per node, node-partition layout: EX_t^T @ ones
            sT_ps = pS.tile([H, T], f32, tag="sT")
            for t in range(T):
                nc.tensor.matmul(
                    sT_ps[:, t:t + 1],
                    ex[0:PK, t * 128:(t + 1) * 128],
                    onesf8[0:PK, :],
                    start=True, stop=True,
                )
            y_sb = mid.tile([GP, T * 128], bf, tag="ysb", bufs=3)
            nc.scalar.activation(y_sb[:], y_ps[:], AF.Copy)
            rS = mid.tile([H, T], f32, tag="rS", bufs=3)
            nc.vector.reciprocal(rS[:], sT_ps[:])
            st.update(y_sb=y_sb, rS=rS)
            return st

        def stageC1(st):
            gi, T = st["gi"], st["T"]
            y_sb, rS = st["y_sb"], st["rS"]
            TH = T * 128
            # c_raw[node, :] = y_t^T @ [W_e^T; b_e]  (+= S*b_e via ones col)
            cr_ps = pC.tile([H, T * HID], f32, tag="cr")
            for t in range(T):
                nc.tensor.matmul(
                    cr_ps[:, t * HID:(t + 1) * HID],
                    y_sb[:, t * 128:(t + 1) * 128], weTa,
                    start=True, stop=True,
                )
            # x = c_raw / S (+ b_e); ctx' = elu(x)+1 = relu(x) + min(exp(x),1)
            x = mid.tile([H, T * HID], bf, tag="x")
            rSb = (
                rS[:].rearrange("p (t o) -> p t o", o=1)
                .broadcast_to([H, T, HID])
            )
            nc.vector.tensor_tensor(
                x[:].rearrange("p (t c) -> p t c", c=HID),
                cr_ps[:].rearrange("p (t c) -> p t c", c=HID),
                rSb, op=OP.mult,
            )
            ev = mid.tile([H, T * HID], bf, tag="ev")
            nc.scalar.activation(ev[:], x[:], AF.Exp)
            rn = mid.tile([H, T * HID], bf, tag="rn")
            nc.gpsimd.tensor_scalar_max(rn[:], x[:], 0.0)
            em = mid.tile([H, T * HID], bf, tag="em")
            nc.gpsimd.tensor_scalar_min(em[:], ev[:], 1.0)
            ctxn = mid.tile([H, T * HID], bf, tag="ctxn")
            nc.vector.tensor_tensor(ctxn[:], em[:], rn[:], op=OP.add)
            # transpose ctx to [HID, node] (ones row persistent in ctxT buf)
            ctxT_ps = pX.tile([HID, TH], bf, tag="ctxTp", bufs=1)
            for t in range(T):
                nc.tensor.transpose(
                    ctxT_ps[:, t * 128:(t + 1) * 128],
                    ctxn[:, t * HID:(t + 1) * HID],
                    identb,
                )
            ctxT = ctxTbufs[st["buf"]]
            nc.vector.tensor_copy(ctxT[0:HID, 0:TH], ctxT_ps[:])
            st["ctxT"] = ctxT
            return st

        def stageC2(st):
            gi, T = st["gi"], st["T"]
            c0 = emit_off[gi] * 128
            TH = T * 128
            ctxT = st["ctxT"]

            # --- GRU gates ---
            t2 = mid.tile([H, TH], bf, tag="t2")
            zq = mid.tile([H, TH], bf, tag="zq")
            sigq = mid.tile([H, 2 * TH], bf, tag="sigq")
            for pair in _pairs(T):
                P2 = len(pair)
                p0 = pair[0]
                rz_ps = pR.tile([H, P2 * 2 * H], f32, tag="rz")
                inhn_ps = pN.tile([H, P2 * 2 * H], f32, tag="inhn")
                for ti, t in enumerate(pair):
                    ctx_t = ctxT[:, t * H:(t + 1) * H]
                    nfT_c = nf_view(nfT_e, nfT_r, c0 + t * H, H)
                    nc.tensor.matmul(
                        rz_ps[:, ti * 2 * H:(ti + 1) * 2 * H],
                        ctx_t, wih[:, 0:2 * H], start=True, stop=False,
                    )
                    nc.tensor.matmul(
                        rz_ps[:, ti * 2 * H:(ti + 1) * 2 * H],
                        nfT_c, whh[:, 0:2 * H], start=False, stop=True,
                    )
                    # in_n = ctx@wih_n (+bias via ones row of ctxT)
                    nc.tensor.matmul(
                        inhn_ps[:, ti * 2 * H:ti * 2 * H + H],
                        ctx_t, wih[:, 2 * H:], start=True, stop=True,
                    )
                    # hn = 0.5*(nf@whh_n) + 0.5*bhh_n
                    nc.tensor.matmul(
                        inhn_ps[:, ti * 2 * H + H:(ti + 1) * 2 * H],
                        nfT_c, whh[:, 2 * H:], start=True, stop=False,
                    )
                    nc.tensor.matmul(
                        inhn_ps[:, ti * 2 * H + H:(ti + 1) * 2 * H],
                        ctxT[HID:HID + 1, t * H:(t + 1) * H], bhhn,
                        start=False, stop=True,
                    )
                # trz = tanh(0.5 * rz); r = (1+tr)/2, z = (1+tz)/2
                sig = sigq[:, p0 * 2 * H:(p0 + P2) * 2 * H]
                nc.scalar.activation(sig, rz_ps[:], AF.Tanh, scale=0.5)
                sigv = sig.rearrange("p (t two h) -> p t two h", two=2, h=H)
                tr_v = sigv[:, :, 0, :]
                # z = 0.5*tz + 0.5 (true sigmoid), off the critical chain
                nc.vector.tensor_scalar(
                    zq[:, p0 * H:(p0 + P2) * H].rearrange("p (t h) -> p t h", h=H),
                    sigv[:, :, 1, :], 0.5, 0.5, op0=OP.mult, op1=OP.add,
                )
                iv = inhn_ps[:].rearrange("p (t two h) -> p t two h", two=2, h=H)
                in_v = iv[:, :, 0, :]
                hn_v = iv[:, :, 1, :]
                # t2 = (tr+1)*hn + in_n  (= i_n + r*(h_n + bhh_n))
                t1 = mid.tile([H, P2 * H], bf, tag="t1")
                nc.vector.scalar_tensor_tensor(
                    t1[:].rearrange("p (t h) -> p t h", h=H),
                    tr_v, 1.0, hn_v, OP.add, OP.mult,
                )
                nc.vector.tensor_tensor(
                    t2[:, p0 * H:(p0 + P2) * H].rearrange("p (t h) -> p t h", h=H),
                    t1[:].rearrange("p (t h) -> p t h", h=H),
                    in_v, op=OP.add,
                )
            # n = tanh(t2); h = relu(n + z*(nf - n))
            n_t = mid.tile([H, TH], bf, tag="n")
            nc.scalar.activation(n_t[:], t2[:], AF.Tanh)
            nf_v = nf_view(nf_e, nf_r, c0, TH)
            d_t = mid.tile([H, TH], bf, tag="d")
            nc.vector.tensor_tensor(d_t[:], nf_v, n_t[:], op=OP.subtract)
            zd = mid.tile([H, TH], bf, tag="zd")
            nc.vector.tensor_tensor(zd[:], zq[:, 0:TH], d_t[:], op=OP.mult)
            s1 = mid.tile([H, TH], bf, tag="s1")
            nc.vector.tensor_tensor(s1[:], n_t[:], zd[:], op=OP.add)
            nc.vector.tensor_scalar_max(ho_all[:, c0:c0 + TH], s1[:], 0.0)

        # --- 4-stage software pipeline over groups:
        # A(g) dma | B(g-1) exp+y | C1(g-2) ctx | C2(g-3) gru ---
        NG = len(gspec)
        stA = stB = stC = None
        bufc = 0
        consumed = 0
        store0_cols = None
        store1_cols = None
        for i in range(NG + 3):
            a = None
            if i < NG:
                with tc.high_priority(offset=1200):
                    a = stageA(order[i])
            if stA is not None:
                with tc.high_priority(offset=600):
                    newB = stageB(stA)
            else:
                newB = None
            if i == 0:
                # node features for the first few emitted groups right after
                # the first pf group; the big remainder rides mid-stream
                with tc.high_priority(offset=5000):
                    nc.sync.dma_start(nfT_e[:], nfT_d[:, 0:SPL])
                    nc.sync.dma_start(nf_e[:], nf_d[:, 0:SPL])
            if i == 4:
                nc.sync.dma_start(nfT_r[:], nfT_d[:, SPL:])
                nc.sync.dma_start(nf_r[:], nf_d[:, SPL:])
            if stB is not None:
                stB["buf"] = bufc % 2
                bufc += 1
                newC = stageC1(stB)
            else:
                newC = None
            if stC is not None:
                stageC2(stC)
                consumed += stC["T"]
            stA, stB, stC = a, newB, newC
            if i == NG - 2 and store0_cols is None:
                store0_cols = consumed * 128
                nc.sync.dma_start(
                    hout_d[:, 0:store0_cols], ho_all[:, 0:store0_cols]
                )
            if i == NG + 1 and store1_cols is None:
                # partial store once all but the last two groups' tails are
                # queued; covers everything already computed
                store1_cols = consumed * 128
                nc.sync.dma_start(
                    hout_d[:, store0_cols:store1_cols],
                    ho_all[:, store0_cols:store1_cols],
                )
        nc.sync.dma_start(
            hout_d[:, store1_cols:], ho_all[:, store1_cols:]
        )
        assert consumed == NT
